# revision 15
# baseline (speedup 1.0000x reference)
"""Bass/Trainium2 kernel for nn_Attention_42305427865835.

Computes, for d_hidden [B,N,D], encoder_outputs [B,Lin,E], W1 [E+N*D, D],
b1 [D], w2 [D]:
    dec_proj = d_flat @ W1[:N*D] + b1                    # [B, D]
    enc_proj = enc @ W1[N*D:]                            # [B, Lin, E->D]
    scores   = tanh(enc_proj + dec_proj[:,None,:]) @ w2  # [B, Lin]
    out      = softmax(scores, axis=-1)

Sharding: data-parallel over batch, 4 batches per core on 8 cores.
Device-side layout is transposed ("T layout": D/E on partitions, Lin on the
free axis) so the contraction over E maps directly onto the PE array and the
dec_proj/b1 bias-add rides the ScalarE activation's per-partition bias.
The host feeds each core its encoder slice pre-transposed (and pre-cast to
bf16) as [BPC, E, Lin], plus two packed bf16 weight tensors.

Matmul operands are bf16 (PSUM accumulation stays fp32): 2-byte weights get
fast-weight-load, and enc DMA bytes halve. The dec bias path and softmax
stay fp32.

TRN2 instructions carry at most one semaphore wait, so the module is built
with bacc.Bacc and finished with nc.compile(), whose
generate_event_semaphores pass splits multi-wait instructions.

Softmax skips the max-subtraction: |scores| <= ||w2||_1 ~ 11, well inside
exp's fp32 range, so it matches the reference within rounding.
"""

import numpy as np

B, LIN, E, D, N = 32, 2048, 512, 512, 2
NCORES = 8
BPC = B // NCORES      # batches per core
P = 128                # SBUF partitions
ETILES = E // P        # 4
DTILES = D // P        # 4
ND = N * D             # 1024
KTILES = ND // P       # 8
LCHW = 512             # Lin chunk width (one PSUM bank of fp32)
LCH = LIN // LCHW      # 4

# packed-weights free-dim layouts (per partition p), all bf16
# wpackA: w1e + dh + b1 + w2 + zero
W1E_OFF = 0
W1E_LEN = ETILES * D           # 2048: [e, d] -> W1_e[e*P+p, d]
DH_OFF = W1E_OFF + W1E_LEN
DH_LEN = KTILES * BPC          # 32:   [k, b] -> d_flat[b, k*P+p]
W2_OFF = DH_OFF + DH_LEN
W2_LEN = DTILES                # 4:    [a]    -> w2[a*P+p]
WPACKA = W2_OFF + W2_LEN       # 2084
# biasz (separate fp32 tensor): b1 columns + a zero column for the Exp bias
BIASZ = DTILES + 1
# wpackB: w1d only
W1D_LEN = KTILES * D           # 4096: [k, d] -> W1_d[k*P+p, d]

TRACE = False
TRACE_KWARGS = {}
LAST_RESULT = None

_CACHE = {}


def _build():
    import concourse.bacc as bacc
    import concourse.mybir as mybir
    import concourse.tile as tile
    from concourse.bass import ts

    f32 = mybir.dt.float32
    bf16 = mybir.dt.bfloat16
    AF = mybir.ActivationFunctionType
    AX = mybir.AxisListType

    nc = bacc.Bacc("TRN2", target_bir_lowering=False)

    encC_h = nc.dram_tensor(
        "encC", [BPC, LCH, P, ETILES, LCHW], bf16, kind="ExternalInput"
    )
    wpackA_h = nc.dram_tensor("wpackA", [P, WPACKA], bf16, kind="ExternalInput")
    wpackB_h = nc.dram_tensor("wpackB", [P, W1D_LEN], bf16, kind="ExternalInput")
    biasz_h = nc.dram_tensor("biasz", [P, BIASZ], f32, kind="ExternalInput")
    out_h = nc.dram_tensor("out", [BPC, LIN], f32, kind="ExternalOutput")

    with tile.TileContext(nc) as tc:
        with (
            tc.tile_pool(name="persist", bufs=1) as wp,
            tc.tile_pool(name="encp", bufs=3 * LCH) as encp,
            tc.tile_pool(name="attnp", bufs=3 * DTILES) as attnp,
            tc.tile_pool(name="smp", bufs=BPC) as smp,
            tc.tile_pool(name="mainps", bufs=6, space="PSUM") as mainps,
            tc.tile_pool(name="scpsp", bufs=1, space="PSUM") as scpsp,
            tc.tile_pool(name="decps", bufs=1, space="PSUM") as decps,
        ):
            # --- weights + first enc chunk first (critical path), rest after ---
            wsbA = wp.tile([P, WPACKA], bf16, tag="wsbA")
            nc.sync.dma_start(out=wsbA, in_=wpackA_h[:, :])

            enc_tiles = [
                [
                    encp.tile(
                        [P, ETILES, LCHW], bf16, tag="enc", name=f"enc_b{b}l{lc}"
                    )
                    for lc in range(LCH)
                ]
                for b in range(BPC)
            ]
            nc.sync.dma_start(out=enc_tiles[0][0], in_=encC_h[0, 0])

            wsbB = wp.tile([P, W1D_LEN], bf16, tag="wsbB")
            nc.sync.dma_start(out=wsbB, in_=wpackB_h[:, :])
            biasz_sb = wp.tile([P, BIASZ], f32, tag="biasz")
            nc.sync.dma_start(out=biasz_sb, in_=biasz_h[:, :])

            w1e_sb = wsbA[:, W1E_OFF : W1E_OFF + W1E_LEN].rearrange(
                "p (e d) -> p e d", e=ETILES
            )
            dh_sb = wsbA[:, DH_OFF : DH_OFF + DH_LEN].rearrange(
                "p (k b) -> p k b", k=KTILES
            )
            b1_sb = biasz_sb[:, 0:DTILES]
            w2_sb = wsbA[:, W2_OFF : W2_OFF + W2_LEN]
            zbias = biasz_sb[0:1, DTILES : DTILES + 1]
            w1d_sb = wsbB.rearrange("p (k d) -> p k d", k=KTILES)

            decb = wp.tile([P, DTILES, BPC], f32, tag="decb")

            # HAM warmup: the PE idles ~6us waiting for the first DMAs and
            # would then ramp from the cold 1.2GHz clock during real work.
            # A dozen dummy matmuls on a zeroed tile (no DMA deps) put the
            # ~3.4us activity ramp inside the DMA wait instead.
            warm = wp.tile([P, LCHW], bf16, tag="warm")
            nc.vector.memset(warm, 0)
            for w in range(12):
                wps = decps.tile([P, LCHW], f32, tag="d", name=f"warmps{w}")
                nc.tensor.matmul(
                    out=wps,
                    lhsT=warm[:, 0:P],
                    rhs=warm,
                    start=True,
                    stop=True,
                )

            def emit_dec():
                # dec_projT + b1 bias columns: [p, dtile, batch]; emitted after
                # batch-0 chunk-0 so these wpackB-gated matmuls don't block the
                # in-order PE queue during the initial DMA
                for j in range(DTILES):
                    dps = decps.tile([P, BPC], f32, tag="d", name=f"decps{j}")
                    for k in range(KTILES):
                        nc.tensor.matmul(
                            out=dps,
                            lhsT=w1d_sb[:, k, ts(j, P)],
                            rhs=dh_sb[:, k, :],
                            start=(k == 0),
                            stop=(k == KTILES - 1),
                        )
                    nc.vector.tensor_scalar_add(
                        out=decb[:, j, :], in0=dps, scalar1=b1_sb[:, j : j + 1]
                    )

            # --- main loop: per batch, enc_projT -> tanh -> w2 dot -> softmax ---
            for b in range(BPC):
                for lc in range(LCH):
                    if b == 0 and lc == 0:
                        continue  # issued up-front
                    nc.sync.dma_start(out=enc_tiles[b][lc], in_=encC_h[b, lc])

                erow = smp.tile([1, LCH, LCHW], f32, tag="erow", name=f"erow{b}")
                sumexps = smp.tile([1, LCH], f32, tag="sumexps", name=f"sumexps{b}")
                for lc in range(LCH):
                    sc = scpsp.tile([1, LCHW], f32, tag="sc", name=f"sc{b}l{lc}")
                    mpss = []
                    for j in range(DTILES):
                        mps = mainps.tile(
                            [P, LCHW], f32, tag="m", name=f"mps_b{b}l{lc}j{j}"
                        )
                        for e in range(ETILES):
                            nc.tensor.matmul(
                                out=mps,
                                lhsT=w1e_sb[:, e, ts(j, P)],
                                rhs=enc_tiles[b][lc][:, e, :],
                                start=(e == 0),
                                stop=(e == ETILES - 1),
                            )
                        mpss.append(mps)
                    if b == 0 and lc == 0:
                        # dec matmuls slot in here: after the first chunk's
                        # main groups (so they don't head-block the in-order
                        # PE queue during the initial DMA) but before the
                        # first tanh, which reads decb
                        emit_dec()
                    attns = []
                    for j in range(DTILES):
                        at = attnp.tile(
                            [P, LCHW], bf16, tag="attn", name=f"attn_b{b}l{lc}j{j}"
                        )
                        nc.scalar.activation(
                            out=at,
                            in_=mpss[j],
                            func=AF.Tanh,
                            bias=decb[:, j, b : b + 1],
                            scale=1.0,
                        )
                        attns.append(at)
                    for j in range(DTILES):
                        nc.tensor.matmul(
                            out=sc,
                            lhsT=w2_sb[:, j : j + 1],
                            rhs=attns[j],
                            start=(j == 0),
                            stop=(j == DTILES - 1),
                        )
                    # exp of this chunk right away (scores are bounded,
                    # |s|<=~11, so no max-subtraction is needed in fp32)
                    nc.scalar.activation(
                        out=erow[:, lc, :],
                        in_=sc,
                        func=AF.Exp,
                        bias=zbias,
                        scale=1.0,
                        accum_out=sumexps[:, lc : lc + 1],
                    )

                sumexp = smp.tile([1, 1], f32, tag="sumexp", name=f"sumexp{b}")
                nc.vector.reduce_sum(out=sumexp, in_=sumexps, axis=AX.X)
                rinv = smp.tile([1, 1], f32, tag="rinv", name=f"rinv{b}")
                nc.vector.reciprocal(out=rinv, in_=sumexp)
                orow = smp.tile([1, LCH, LCHW], f32, tag="orow", name=f"orow{b}")
                nc.vector.tensor_scalar_mul(out=orow, in0=erow, scalar1=rinv)
                nc.sync.dma_start(
                    out=out_h[b : b + 1, :], in_=orow.rearrange("p a b -> p (a b)")
                )
    nc.compile()
    return nc


def _pack_weights(W1, b1, w2, dhT):
    """Build the (wpackA, wpackB) bf16 arrays for one core."""
    import ml_dtypes

    bf = ml_dtypes.bfloat16
    W1d = W1[:ND]                       # [ND, D]
    W1e = W1[ND:]                       # [E, D]
    wpackA = np.zeros((P, WPACKA), dtype=bf)
    wpackA[:, W1E_OFF : W1E_OFF + W1E_LEN] = (
        W1e.reshape(ETILES, P, D).transpose(1, 0, 2).reshape(P, W1E_LEN).astype(bf)
    )
    wpackA[:, DH_OFF : DH_OFF + DH_LEN] = (
        dhT.reshape(KTILES, P, BPC).transpose(1, 0, 2).reshape(P, DH_LEN).astype(bf)
    )
    wpackA[:, W2_OFF : W2_OFF + W2_LEN] = w2.reshape(DTILES, P).T.astype(bf)
    wpackB = np.ascontiguousarray(
        W1d.reshape(KTILES, P, D).transpose(1, 0, 2).reshape(P, W1D_LEN).astype(bf)
    )
    biasz = np.zeros((P, BIASZ), dtype=np.float32)
    biasz[:, 0:DTILES] = b1.reshape(DTILES, P).T
    return wpackA, wpackB, biasz


def _prep_in_maps(d_hidden, encoder_outputs, W1, b1, w2):
    import ml_dtypes

    bf = ml_dtypes.bfloat16
    d_hidden = np.ascontiguousarray(np.asarray(d_hidden), dtype=np.float32)
    encoder_outputs = np.ascontiguousarray(
        np.asarray(encoder_outputs), dtype=np.float32
    )
    W1 = np.ascontiguousarray(np.asarray(W1), dtype=np.float32)
    b1 = np.ascontiguousarray(np.asarray(b1), dtype=np.float32)
    w2 = np.ascontiguousarray(np.asarray(w2), dtype=np.float32)

    in_maps = []
    for c in range(NCORES):
        bs = slice(c * BPC, (c + 1) * BPC)
        encT = encoder_outputs[bs].transpose(0, 2, 1)  # [BPC, E, LIN]
        encC = np.ascontiguousarray(
            encT.reshape(BPC, ETILES, P, LCH, LCHW)
            .transpose(0, 3, 2, 1, 4)
            .astype(bf)
        )
        dhT = np.ascontiguousarray(d_hidden[bs].reshape(BPC, ND).T)
        wpackA, wpackB, biasz = _pack_weights(W1, b1, w2, dhT)
        in_maps.append(
            {"encC": encC, "wpackA": wpackA, "wpackB": wpackB, "biasz": biasz}
        )
    return in_maps


def kernel(d_hidden, encoder_outputs, W1, b1, w2):
    global LAST_RESULT
    from concourse import bass_utils

    if "nc" not in _CACHE:
        _CACHE["nc"] = _build()
    nc = _CACHE["nc"]

    in_maps = _prep_in_maps(d_hidden, encoder_outputs, W1, b1, w2)
    res = bass_utils.run_bass_kernel_spmd(
        nc,
        in_maps,
        core_ids=list(range(NCORES)),
        trace=TRACE,
        **TRACE_KWARGS,
    )
    LAST_RESULT = res
    return np.concatenate([r["out"] for r in res.results], axis=0)
